# revision 10
# baseline (speedup 1.0000x reference)
"""BinaryLinear (sign-binarized weight linear layer) on 8 Trainium2 NeuronCores.

y[b,s,o] = sum_i x[b,s,i] * (scale[o] * sign(w[o,i])) + bias[o]
  with scale[o] = mean_i |w[o,i]|

Sharding: data-parallel over the batch dim (8 batches -> 8 cores); w/bias
replicated. All reference MATH (sign, scale, matmul, bias) runs on device;
the host only re-lays-out / casts inputs so the device pipeline is pure
2D line-rate DMA loads with zero on-device transposes:
  - x[b].T staged bf16 as 4 s-chunks [P, KT*512] (2D per-partition rows)
  - w staged twice in fp8e4 (sign-exact: |w| clamped to the e4m3 subnormal
    floor 2^-9 on host so no weight rounds to 0): o-blocked transpose for
    the matmul lhsT path, row-major for the DVE |w| scale reduce. fp8
    halves the dominant HBM traffic (measured ~131 GB/s/core effective
    when all 8 cores stream).

Compute per core: the first FT=12 of 32 k-subtiles run as fp8 DoubleRow
matmul pairs (2 k-rows per PE cell, ~1.44x bf16 rate; x quantized bf16->
e4m3 on DVE), the rest in bf16; PSUM accumulates f32 across both. Whole-
tensor rel err vs the f32 reference is 1.65e-2 (sim-exact: +-1 x e4m3
products are exact in the PE's e6m3/e10m10 path), under the 2e-2 gate.
ACT Sign slices bt (bf16) + bt8 (fp8 pair-layout) from each wtb block;
DVE fuses psum*scale+bias on eviction, emitting bf16 yT.

Queues: sync = x chunk 0 + bias + output stores; gpsimd = x chunks 1-3 +
cleanup wtb reloads; scalar = wtb/w-row streams. Graduated warm-up: early
o-blocks run only the s-chunks already landed (DMA is co-critical with the
PE at 8-core HBM rates); skipped chunks run at the end from re-signed
reloads, as in the XBAR-era kernel.
"""

import numpy as np
import ml_dtypes

B_DIM = 8
S_DIM = 2048
IN_F = 4096
OUT_F = 4096
P = 128
N_CORES = 8
N_TILE = 512
NCH = S_DIM // N_TILE  # 4 s-chunks
KT = IN_F // P  # 32 k-subtiles
PO = OUT_F // P  # 32 o-blocks
DB = OUT_F // 256  # 16 o double-blocks
FT = 16  # k-subtiles in fp8 DoubleRow pairs
BT = KT - FT  # k-subtiles in bf16

_BUILT = None


def _build_nc():
    from contextlib import ExitStack

    import concourse.mybir as mybir
    import concourse.tile as tile
    from concourse import bacc
    from concourse.bass import ts

    f32 = mybir.dt.float32
    bf16 = mybir.dt.bfloat16
    fp8 = mybir.dt.float8e4
    DR = mybir.MatmulPerfMode.DoubleRow

    nc = bacc.Bacc(None, target_bir_lowering=False, debug=False)
    with tile.TileContext(nc) as tc:
        xtb_d = nc.dram_tensor("xtb", (NCH, P, KT * N_TILE), bf16, kind="ExternalInput")
        wtb_d = nc.dram_tensor("wtb", (DB, P, KT * 256), fp8, kind="ExternalInput")
        w_d = nc.dram_tensor("w", (OUT_F, IN_F), fp8, kind="ExternalInput")
        b_d = nc.dram_tensor("bias", (OUT_F,), f32, kind="ExternalInput")
        yT_d = nc.dram_tensor("yT", (OUT_F, S_DIM), bf16, kind="ExternalOutput")

        with ExitStack() as ctx:
            yT3 = yT_d[:, :].rearrange("(po pi) s -> pi po s", pi=P)

            const = ctx.enter_context(tc.tile_pool(name="const", bufs=1))
            xT = const.tile([P, NCH, BT, N_TILE], bf16)  # resident x^T (bf16 kt)
            xT8 = const.tile([P, NCH, FT // 2, 2, N_TILE], fp8)  # fp8 kt pairs
            scale_sb = const.tile([P, PO], f32)
            bias_sb = const.tile([P, PO], f32)

            wtbpool = ctx.enter_context(tc.tile_pool(name="wtbpool", bufs=2))
            cleanpool = ctx.enter_context(tc.tile_pool(name="cleanpool", bufs=3))
            btpool = ctx.enter_context(tc.tile_pool(name="btpool", bufs=3))
            bt8pool = ctx.enter_context(tc.tile_pool(name="bt8pool", bufs=3))
            xfpool = ctx.enter_context(tc.tile_pool(name="xfpool", bufs=2))
            wrpool = ctx.enter_context(tc.tile_pool(name="wrpool", bufs=2))
            opool = ctx.enter_context(tc.tile_pool(name="opool", bufs=7))
            psum = ctx.enter_context(tc.tile_pool(name="psum", bufs=6, space="PSUM"))

            # ---- load emitters ----
            wtb_tiles = {}

            def load_wtb(db, gen=0):
                pool = cleanpool if gen else wtbpool
                t = pool.tile([P, KT, 256], fp8, tag="wtb", name=f"wtb_{db}_{gen}")
                (nc.gpsimd if gen else nc.scalar).dma_start(t[:], wtb_d[db])
                wtb_tiles[db] = t

            wrow_tiles = {}

            def load_wrow(m):
                t = wrpool.tile([P, IN_F], fp8, tag="wr", name=f"wr_{m}")
                nc.scalar.dma_start(t[:], w_d[ts(m, P), :])
                wrow_tiles[m] = t

            H = FT // 2

            def load_x(n):
                xf = xfpool.tile([P, FT, N_TILE], bf16, tag="xf", name=f"xf_{n}")
                if n == 0:
                    # chunk 0 gates the first matmul: split the fp8 stage
                    # across two rings and cast per half
                    nc.sync.dma_start(xf[:, 0:H, :], xtb_d[n][:, 0 : H * N_TILE])
                    nc.gpsimd.dma_start(
                        xf[:, H:FT, :], xtb_d[n][:, H * N_TILE : FT * N_TILE]
                    )
                    nc.vector.tensor_scalar_mul(
                        xT8[:, n, 0 : H // 2, :, :], xf[:, 0:H, :], 1.0
                    )
                    nc.vector.tensor_scalar_mul(
                        xT8[:, n, H // 2 :, :, :], xf[:, H:FT, :], 1.0
                    )
                else:
                    nc.gpsimd.dma_start(xf[:], xtb_d[n][:, 0 : FT * N_TILE])
                    nc.vector.tensor_scalar_mul(xT8[:, n, :, :, :], xf[:], 1.0)
                nc.sync.dma_start(xT[:, n, :, :], xtb_d[n][:, FT * N_TILE :])

            def make_bt(m):
                db, half = m // 2, m % 2
                src = wtb_tiles[db]
                bt8 = bt8pool.tile([P, FT // 2, 2, P], fp8, tag="bt8", name=f"bt8_{m}")
                bt = btpool.tile([P, BT, P], bf16, tag="bt", name=f"bt_{m}")
                nc.scalar.sign(bt8[:], src[:, 0:FT, ts(half, P)])
                nc.scalar.sign(bt[:], src[:, FT:KT, ts(half, P)])
                if half == 1:
                    wtb_tiles.pop(db)
                return bt8, bt

            def make_scale(m):
                w_sb = wrow_tiles.pop(m)
                nc.vector.tensor_reduce(
                    scale_sb[:, m : m + 1],
                    w_sb[:],
                    axis=mybir.AxisListType.X,
                    op=mybir.AluOpType.add,
                    apply_absolute_value=True,
                )
                nc.vector.tensor_scalar_mul(
                    scale_sb[:, m : m + 1], scale_sb[:, m : m + 1], 1.0 / IN_F
                )

            def mm_block(bts, m, n):
                bt8, bt = bts
                ps = psum.tile([P, N_TILE], f32, name="ps")
                for t8 in range(FT // 2):
                    nc.tensor.matmul(
                        ps[:],
                        bt8[:, t8, :, :],
                        xT8[:, n, t8, :, :],
                        start=(t8 == 0),
                        stop=False,
                        perf_mode=DR,
                    )
                for kt in range(BT):
                    nc.tensor.matmul(
                        ps[:],
                        bt[:, kt, :],
                        xT[:, n, kt, :],
                        start=False,
                        stop=(kt == BT - 1),
                    )
                ob = opool.tile([P, N_TILE], bf16)
                nc.vector.tensor_scalar(
                    ob[:],
                    ps[:],
                    scale_sb[:, m : m + 1],
                    bias_sb[:, m : m + 1],
                    op0=mybir.AluOpType.mult,
                    op1=mybir.AluOpType.add,
                )
                nc.gpsimd.dma_start(yT3[:, m, ts(n, N_TILE)], ob[:])

            # ---- emission ----
            GRAD = [(0, 4, 1), (4, 6, 2), (6, 9, 3), (9, PO, NCH)]
            CLEAN = [(0, 4, 1), (4, 6, 2), (6, 9, 3)]

            load_wtb(0)
            load_x(0)
            load_wrow(0)
            nc.sync.dma_start(bias_sb[:], b_d[:].rearrange("(po pi) -> pi po", pi=P))
            load_x(1)
            load_wrow(1)
            load_x(2)
            load_x(3)

            next_wrow = 2
            for o0, o1, nct in GRAD:
                for m in range(o0, o1):
                    if m % 2 == 0 and m // 2 + 1 < DB:
                        load_wtb(m // 2 + 1)
                    if m == 12:
                        for db in range(3):
                            load_wtb(db, gen=1)
                    bts = make_bt(m)
                    make_scale(m)
                    for _ in range(2):
                        if next_wrow < PO:
                            load_wrow(next_wrow)
                            next_wrow += 1
                    for n in range(nct):
                        mm_block(bts, m, n)
            # cleanup: chunks the warm-up skipped (re-signed from reloads;
            # db 0-2 were staged at m=12, db 3-4 reload here on the idle ring)
            for db in range(3, 5):
                load_wtb(db, gen=1)
            for o0, o1, nct in CLEAN:
                for m in range(o0, o1):
                    bts = make_bt(m)
                    for n in range(nct, NCH):
                        mm_block(bts, m, n)
    nc.finalize()
    return nc


def _get_nc():
    global _BUILT
    if _BUILT is None:
        _BUILT = _build_nc()
    return _BUILT


def _prep_inputs(x, weight, bias):
    bf16 = ml_dtypes.bfloat16
    e4m3 = ml_dtypes.float8_e4m3
    # clamp |w| to the e4m3 subnormal floor so no sign is lost in the cast
    wc = np.copysign(np.maximum(np.abs(weight), 2.0**-9), weight)
    w8 = wc.astype(e4m3)
    # wtb[db, ki, kt*256+oj] = w[db*256+oj, kt*128+ki]  (2D per block)
    wtb = np.ascontiguousarray(
        w8.reshape(DB, 256, KT, P).transpose(0, 3, 2, 1)
    ).reshape(DB, P, KT * 256)
    bias = np.ascontiguousarray(bias, dtype=np.float32)
    per_core = []
    for b in range(N_CORES):
        # xtb[n, ki, kt*512+sj] = x[b, n*512+sj, kt*128+ki]  (2D per chunk)
        xtb = np.ascontiguousarray(
            x[b].T.astype(bf16).reshape(KT, P, NCH, N_TILE).transpose(2, 1, 0, 3)
        ).reshape(NCH, P, KT * N_TILE)
        per_core.append({"xtb": xtb, "wtb": wtb, "w": w8, "bias": bias})
    return per_core


def kernel(x, weight, bias):
    from concourse.bass_utils import run_bass_kernel_spmd

    x = np.asarray(x, dtype=np.float32)
    weight = np.asarray(weight, dtype=np.float32)
    bias = np.asarray(bias, dtype=np.float32)
    assert x.shape == (B_DIM, S_DIM, IN_F), x.shape

    nc = _get_nc()
    in_maps = _prep_inputs(x, weight, bias)
    res = run_bass_kernel_spmd(nc, in_maps, core_ids=list(range(N_CORES)))
    out = np.empty((B_DIM, S_DIM, OUT_F), dtype=np.float32)
    for b in range(N_CORES):
        out[b] = res.results[b]["yT"].astype(np.float32).T
    return out


# revision 11
# speedup vs baseline: 1.0060x; 1.0060x over previous
"""BinaryLinear (sign-binarized weight linear layer) on 8 Trainium2 NeuronCores.

y[b,s,o] = sum_i x[b,s,i] * (scale[o] * sign(w[o,i])) + bias[o]
  with scale[o] = mean_i |w[o,i]|

Sharding: data-parallel over the batch dim (8 batches -> 8 cores); w/bias
replicated. All reference MATH (sign, scale, matmul, bias) runs on device;
the host only re-lays-out / casts inputs so the device pipeline is pure
2D line-rate DMA loads with zero on-device transposes:
  - x[b].T staged bf16 as 4 s-chunks [P, KT*512] (2D per-partition rows)
  - w staged twice in fp8e4 (sign-exact: |w| clamped to the e4m3 subnormal
    floor 2^-9 on host so no weight rounds to 0): o-blocked transpose for
    the matmul lhsT path, row-major for the DVE |w| scale reduce. fp8
    halves the dominant HBM traffic (measured ~131 GB/s/core effective
    when all 8 cores stream).

Compute per core: the first FT=12 of 32 k-subtiles run as fp8 DoubleRow
matmul pairs (2 k-rows per PE cell, ~1.44x bf16 rate; x quantized bf16->
e4m3 on DVE), the rest in bf16; PSUM accumulates f32 across both. Whole-
tensor rel err vs the f32 reference is 1.65e-2 (sim-exact: +-1 x e4m3
products are exact in the PE's e6m3/e10m10 path), under the 2e-2 gate.
ACT Sign slices bt (bf16) + bt8 (fp8 pair-layout) from each wtb block;
DVE fuses psum*scale+bias on eviction, emitting bf16 yT.

Queues: sync = x chunk 0 + bias + output stores; gpsimd = x chunks 1-3 +
cleanup wtb reloads; scalar = wtb/w-row streams. Graduated warm-up: early
o-blocks run only the s-chunks already landed (DMA is co-critical with the
PE at 8-core HBM rates); skipped chunks run at the end from re-signed
reloads, as in the XBAR-era kernel.
"""

import numpy as np
import ml_dtypes

B_DIM = 8
S_DIM = 2048
IN_F = 4096
OUT_F = 4096
P = 128
N_CORES = 8
N_TILE = 512
NCH = S_DIM // N_TILE  # 4 s-chunks
KT = IN_F // P  # 32 k-subtiles
PO = OUT_F // P  # 32 o-blocks
DB = OUT_F // 256  # 16 o double-blocks
FT = 16  # k-subtiles in fp8 DoubleRow pairs
BT = KT - FT  # k-subtiles in bf16

_BUILT = None


def _build_nc():
    from contextlib import ExitStack

    import concourse.mybir as mybir
    import concourse.tile as tile
    from concourse import bacc
    from concourse.bass import ts

    f32 = mybir.dt.float32
    bf16 = mybir.dt.bfloat16
    fp8 = mybir.dt.float8e4
    DR = mybir.MatmulPerfMode.DoubleRow

    nc = bacc.Bacc(None, target_bir_lowering=False, debug=False)
    with tile.TileContext(nc) as tc:
        xtb_d = nc.dram_tensor("xtb", (NCH, P, KT * N_TILE), bf16, kind="ExternalInput")
        wtb_d = nc.dram_tensor("wtb", (DB, P, KT * 256), fp8, kind="ExternalInput")
        w_d = nc.dram_tensor("w", (OUT_F, IN_F), fp8, kind="ExternalInput")
        b_d = nc.dram_tensor("bias", (OUT_F,), f32, kind="ExternalInput")
        yT_d = nc.dram_tensor("yT", (OUT_F, S_DIM), bf16, kind="ExternalOutput")

        with ExitStack() as ctx:
            yT3 = yT_d[:, :].rearrange("(po pi) s -> pi po s", pi=P)

            const = ctx.enter_context(tc.tile_pool(name="const", bufs=1))
            xT = const.tile([P, NCH, BT, N_TILE], bf16)  # resident x^T (bf16 kt)
            xT8 = const.tile([P, NCH, FT // 2, 2, N_TILE], fp8)  # fp8 kt pairs
            scale_sb = const.tile([P, PO], f32)
            bias_sb = const.tile([P, PO], f32)

            wtbpool = ctx.enter_context(tc.tile_pool(name="wtbpool", bufs=2))
            cleanpool = ctx.enter_context(tc.tile_pool(name="cleanpool", bufs=3))
            btpool = ctx.enter_context(tc.tile_pool(name="btpool", bufs=3))
            bt8pool = ctx.enter_context(tc.tile_pool(name="bt8pool", bufs=3))
            xfpool = ctx.enter_context(tc.tile_pool(name="xfpool", bufs=2))
            wrpool = ctx.enter_context(tc.tile_pool(name="wrpool", bufs=2))
            opool = ctx.enter_context(tc.tile_pool(name="opool", bufs=7))
            psum = ctx.enter_context(tc.tile_pool(name="psum", bufs=6, space="PSUM"))

            # ---- load emitters ----
            wtb_tiles = {}

            def load_wtb(db, gen=0):
                pool = cleanpool if gen else wtbpool
                t = pool.tile([P, KT, 256], fp8, tag="wtb", name=f"wtb_{db}_{gen}")
                (nc.gpsimd if gen else nc.scalar).dma_start(t[:], wtb_d[db])
                wtb_tiles[db] = t

            wrow_tiles = {}

            def load_wrow(m):
                t = wrpool.tile([P, IN_F], fp8, tag="wr", name=f"wr_{m}")
                nc.scalar.dma_start(t[:], w_d[ts(m, P), :])
                wrow_tiles[m] = t

            H = FT // 2

            def load_xf(n):
                xf = xfpool.tile([P, FT, N_TILE], bf16, tag="xf", name=f"xf_{n}")
                if n == 0:
                    # chunk 0 gates the first matmul: split the fp8 stage
                    # across two rings and cast per half
                    nc.sync.dma_start(xf[:, 0:H, :], xtb_d[n][:, 0 : H * N_TILE])
                    nc.gpsimd.dma_start(
                        xf[:, H:FT, :], xtb_d[n][:, H * N_TILE : FT * N_TILE]
                    )
                    nc.vector.tensor_scalar_mul(
                        xT8[:, n, 0 : H // 2, :, :], xf[:, 0:H, :], 1.0
                    )
                    nc.vector.tensor_scalar_mul(
                        xT8[:, n, H // 2 :, :, :], xf[:, H:FT, :], 1.0
                    )
                else:
                    nc.gpsimd.dma_start(xf[:], xtb_d[n][:, 0 : FT * N_TILE])
                    nc.vector.tensor_scalar_mul(xT8[:, n, :, :, :], xf[:], 1.0)

            def load_xbf(n):
                nc.sync.dma_start(xT[:, n, :, :], xtb_d[n][:, FT * N_TILE :])

            def make_bt(m):
                db, half = m // 2, m % 2
                src = wtb_tiles[db]
                bt8 = bt8pool.tile([P, FT // 2, 2, P], fp8, tag="bt8", name=f"bt8_{m}")
                bt = btpool.tile([P, BT, P], bf16, tag="bt", name=f"bt_{m}")
                nc.scalar.sign(bt8[:], src[:, 0:FT, ts(half, P)])
                nc.scalar.sign(bt[:], src[:, FT:KT, ts(half, P)])
                if half == 1:
                    wtb_tiles.pop(db)
                return bt8, bt

            def make_scale(m):
                w_sb = wrow_tiles.pop(m)
                nc.vector.tensor_reduce(
                    scale_sb[:, m : m + 1],
                    w_sb[:],
                    axis=mybir.AxisListType.X,
                    op=mybir.AluOpType.add,
                    apply_absolute_value=True,
                )
                nc.vector.tensor_scalar_mul(
                    scale_sb[:, m : m + 1], scale_sb[:, m : m + 1], 1.0 / IN_F
                )

            def mm_block(bts, m, n):
                bt8, bt = bts
                ps = psum.tile([P, N_TILE], f32, name="ps")
                for t8 in range(FT // 2):
                    nc.tensor.matmul(
                        ps[:],
                        bt8[:, t8, :, :],
                        xT8[:, n, t8, :, :],
                        start=(t8 == 0),
                        stop=False,
                        perf_mode=DR,
                    )
                for kt in range(BT):
                    nc.tensor.matmul(
                        ps[:],
                        bt[:, kt, :],
                        xT[:, n, kt, :],
                        start=False,
                        stop=(kt == BT - 1),
                    )
                ob = opool.tile([P, N_TILE], bf16)
                nc.vector.tensor_scalar(
                    ob[:],
                    ps[:],
                    scale_sb[:, m : m + 1],
                    bias_sb[:, m : m + 1],
                    op0=mybir.AluOpType.mult,
                    op1=mybir.AluOpType.add,
                )
                nc.gpsimd.dma_start(yT3[:, m, ts(n, N_TILE)], ob[:])

            # ---- emission ----
            GRAD = [(0, 4, 1), (4, 6, 2), (6, 9, 3), (9, PO, NCH)]
            CLEAN = [(0, 4, 1), (4, 6, 2), (6, 9, 3)]

            nc.scalar.dma_start(
                bias_sb[:], b_d[:].rearrange("(po pi) -> pi po", pi=P)
            )
            load_wtb(0)
            load_xf(0)
            load_xbf(0)
            load_wrow(0)
            load_xf(1)
            load_xbf(1)
            load_wrow(1)
            load_xf(2)
            load_xbf(2)
            load_xf(3)
            load_xbf(3)

            next_wrow = 2
            for o0, o1, nct in GRAD:
                for m in range(o0, o1):
                    if m % 2 == 0 and m // 2 + 1 < DB:
                        load_wtb(m // 2 + 1)
                    if m == 12:
                        for db in range(3):
                            load_wtb(db, gen=1)
                    bts = make_bt(m)
                    make_scale(m)
                    for _ in range(2):
                        if next_wrow < PO:
                            load_wrow(next_wrow)
                            next_wrow += 1
                    for n in range(nct):
                        mm_block(bts, m, n)
            # cleanup: chunks the warm-up skipped (re-signed from reloads;
            # db 0-2 were staged at m=12, db 3-4 reload here on the idle ring)
            for db in range(3, 5):
                load_wtb(db, gen=1)
            for o0, o1, nct in CLEAN:
                for m in range(o0, o1):
                    bts = make_bt(m)
                    for n in range(nct, NCH):
                        mm_block(bts, m, n)
    nc.finalize()
    return nc


def _get_nc():
    global _BUILT
    if _BUILT is None:
        _BUILT = _build_nc()
    return _BUILT


def _prep_inputs(x, weight, bias):
    bf16 = ml_dtypes.bfloat16
    e4m3 = ml_dtypes.float8_e4m3
    # clamp |w| to the e4m3 subnormal floor so no sign is lost in the cast
    wc = np.copysign(np.maximum(np.abs(weight), 2.0**-9), weight)
    w8 = wc.astype(e4m3)
    # wtb[db, ki, kt*256+oj] = w[db*256+oj, kt*128+ki]  (2D per block)
    wtb = np.ascontiguousarray(
        w8.reshape(DB, 256, KT, P).transpose(0, 3, 2, 1)
    ).reshape(DB, P, KT * 256)
    bias = np.ascontiguousarray(bias, dtype=np.float32)
    per_core = []
    for b in range(N_CORES):
        # xtb[n, ki, kt*512+sj] = x[b, n*512+sj, kt*128+ki]  (2D per chunk)
        xtb = np.ascontiguousarray(
            x[b].T.astype(bf16).reshape(KT, P, NCH, N_TILE).transpose(2, 1, 0, 3)
        ).reshape(NCH, P, KT * N_TILE)
        per_core.append({"xtb": xtb, "wtb": wtb, "w": w8, "bias": bias})
    return per_core


def kernel(x, weight, bias):
    from concourse.bass_utils import run_bass_kernel_spmd

    x = np.asarray(x, dtype=np.float32)
    weight = np.asarray(weight, dtype=np.float32)
    bias = np.asarray(bias, dtype=np.float32)
    assert x.shape == (B_DIM, S_DIM, IN_F), x.shape

    nc = _get_nc()
    in_maps = _prep_inputs(x, weight, bias)
    res = run_bass_kernel_spmd(nc, in_maps, core_ids=list(range(N_CORES)))
    out = np.empty((B_DIM, S_DIM, OUT_F), dtype=np.float32)
    for b in range(N_CORES):
        out[b] = res.results[b]["yT"].astype(np.float32).T
    return out
